# revision 16
# baseline (speedup 1.0000x reference)
"""GTN (graph transformer network) meta-path kernel for TRN2, 8 NeuronCores.

Math (reference):
    Ap = A transposed to [E, N, N]
    a  = sum_e softmax(w1_0)[c,e] * Ap[e]      (per channel c)
    b  = sum_e softmax(w2_0)[c,e] * Ap[e]
    H  = a @ b
    twice:  H = normalize(H) @ gtconv(Ap, w)   (normalize = zero diag, col-scale)
    out = symmetrized mean over channels.

Sharding: channel-parallel — core c computes channel c end to end, then the
symmetrized per-channel result is summed with one bf16 ReduceScatter; core c
returns output rows [256c:256c+256] and the host concatenates the bands.

On-device formulation works with TRANSPOSED intermediates Ht = H^T so that
 - every GEMM's moving operand is the previous GEMM's output as-is,
 - normalization becomes row sums (free-dim reduce) + per-partition scale.

All GEMMs run in fp8(e4m3) DoubleRow perf mode (256-deep contraction per
pass, ~1.7x bf16 PE throughput). The normalized operand is scaled by
SCALE=1024 when quantized to fp8 (entries land ~0.5); normalization is
scale-invariant so the factor self-cancels at the next normalize, and the
final GEMM's output stage divides it back out along with the channel mean.

The edge-type mix contracts e on SBUF partitions: the host feeds A once in
fp8 layout At3[b, (k16 e), j] and a block-diagonal DoubleRow weight computes
all four mixes for a PAIR of 16-row blocks per pass. PSUM tiles are copied
full-width to bf16 staging; the a-plane rows go to a bf16 DRAM plane (read
back with DMA-transpose to build GEMM1's moving operand a^T) and the
b/g1/g2 rows go to fp8 DRAM planes via one casting SWDGE DMA per block
group. GEMM stationaries are SBUF-resident for the whole run.
"""

import numpy as np

N = 2048
E = 8
C = 8
P = 128
NCORES = 8
SCALE = 1024.0

_PROGRAM = None


def _softmax_rows(w: np.ndarray) -> np.ndarray:
    """w: [C, E, 1, 1] -> softmax over E, float64 precision, returns [C, E]."""
    x = w.reshape(C, E).astype(np.float64)
    x = x - x.max(axis=1, keepdims=True)
    ex = np.exp(x)
    return ex / ex.sum(axis=1, keepdims=True)


def _build_program():
    import concourse.bacc as bacc
    import concourse.mybir as mybir
    import concourse.tile as tile

    f32 = mybir.dt.float32
    bf16 = mybir.dt.bfloat16
    f8 = mybir.dt.float8e4
    AX = mybir.AxisListType.X
    MUL = mybir.AluOpType.mult
    ADD = mybir.AluOpType.add
    NE = mybir.AluOpType.not_equal
    COPY = mybir.ActivationFunctionType.Copy
    DR = mybir.MatmulPerfMode.DoubleRow

    nc = bacc.Bacc("TRN2")
    A3_ext = nc.dram_tensor("At3", [P, P, N], f8, kind="ExternalInput")
    w4_ext = nc.dram_tensor("w4d", [P, 2, P], f8, kind="ExternalInput")
    out_ext = nc.dram_tensor("out", [2 * P, N], f32, kind="ExternalOutput")

    with tile.TileContext(nc) as tc:
        with (
            tc.tile_pool(name="dram", bufs=1, space="DRAM") as dpool,
            tc.tile_pool(name="const", bufs=1) as cpool,
        ):
            aplane = dpool.tile([N, N], bf16)        # 'a' mix, natural [i, j]
            nat3 = dpool.tile([3, N, N], f8)         # b/g1/g2 mixes, natural
            h2t = dpool.tile([N, N], bf16)           # H''^T
            ysym = dpool.tile([N, N], bf16)          # H'' + H''^T (per channel)
            rs = dpool.tile([2 * P, N], bf16)        # ReduceScatter shard

            # --- constants ---
            w4_sb = cpool.tile([P, 2, P], f8)
            nc.sync.dma_start(out=w4_sb[:], in_=w4_ext[:])
            # diag masks: masks[:, v, y] = 0 where y == p + v*128 else 1
            masks = cpool.tile([P, 4, 512], f32)
            nc.gpsimd.memset(masks[:], 1.0)
            for v in range(4):
                nc.gpsimd.affine_select(
                    out=masks[:, v],
                    in_=masks[:, v],
                    compare_op=NE,
                    fill=0.0,
                    base=v * P,
                    pattern=[[-1, 512]],
                    channel_multiplier=1,
                )

            with tc.tile_pool(name="big", bufs=1) as bigpool:
                mv = [
                    bigpool.tile([P, 16, N], f8, tag="mv0", name="mva"),
                    bigpool.tile([P, 16, N], f8, tag="mv1", name="mvb"),
                ]

                # ======= Phase 1: all four mixes, DoubleRow block pairs ====
                with (
                    tc.tile_pool(name="mix", bufs=3) as mpool,
                    tc.tile_pool(name="mixst", bufs=3) as spool,
                    tc.tile_pool(name="mvt", bufs=2) as tpool,
                    tc.tile_pool(name="mixps", bufs=1, space="PSUM") as mpsum,
                ):
                    for ld4 in range(32):
                        a3t = mpool.tile([P, 4, N], f8, tag="a3t")
                        nc.sync.dma_start(
                            out=a3t[:],
                            in_=A3_ext[4 * ld4 : 4 * ld4 + 4].rearrange(
                                "b p j -> p b j"
                            ),
                        )
                        st8 = spool.tile([P, 2, N], f8, tag="st8")
                        sta = spool.tile([32, 2, N], bf16, tag="sta")
                        for h in range(2):
                            for jc in range(4):
                                pm = mpsum.tile(
                                    [P, 512], f32, tag=f"pm{h}{jc}", name="pm"
                                )
                                nc.tensor.matmul(
                                    pm[:],
                                    lhsT=w4_sb[:],
                                    rhs=a3t[
                                        :,
                                        2 * h : 2 * h + 2,
                                        jc * 512 : (jc + 1) * 512,
                                    ],
                                    start=True,
                                    stop=True,
                                    perf_mode=DR,
                                )
                                nc.vector.tensor_copy(
                                    out=st8[:, h, jc * 512 : (jc + 1) * 512],
                                    in_=pm[:],
                                )
                                nc.scalar.copy(
                                    sta[:, h, jc * 512 : (jc + 1) * 512],
                                    pm[0:32, :],
                                )
                        # a rows -> bf16 plane (1 DMA); SBUF AP stays
                        # partition-outermost, DRAM side reordered to match
                        nc.scalar.dma_start(
                            out=aplane[64 * ld4 : 64 * ld4 + 64, :].rearrange(
                                "(h r) j -> r h j", h=2
                            ),
                            in_=sta[:],
                        )
                        # b/g1/g2 rows -> fp8 planes (3 DMAs)
                        for q in range(3):
                            eng = (nc.sync, nc.gpsimd, nc.scalar)[q]
                            eng.dma_start(
                                out=nat3[
                                    q, 64 * ld4 : 64 * ld4 + 64, :
                                ].rearrange("(h r) j -> r h j", h=2),
                                in_=st8[32 * (q + 1) : 32 * (q + 2)],
                            )
                    # mv0 = a^T: DMA-transpose columns, cast bf16 -> fp8
                    for kc in range(16):
                        mt = tpool.tile([P, N], bf16, tag="mt")
                        teng = nc.sync if kc % 2 == 0 else nc.scalar
                        teng.dma_start_transpose(
                            out=mt[:], in_=aplane[:, kc * P : (kc + 1) * P]
                        )
                        if kc % 2 == 0:
                            nc.vector.tensor_copy(out=mv[0][:, kc, :], in_=mt[:])
                        else:
                            nc.scalar.copy(mv[0][:, kc, :], mt[:])
                # =========== Phases 2-4: three chained GEMMs ===========
                with (
                    tc.tile_pool(name="plns", bufs=1) as ppool,
                    tc.tile_pool(name="gw", bufs=2) as gpool,
                    tc.tile_pool(name="nrm", bufs=4) as npool,
                    tc.tile_pool(name="gps", bufs=2, space="PSUM") as gpsum,
                ):
                    # resident stationary plane loads (b blocks GEMM1;
                    # g1/g2 overlap GEMM1/2)
                    planes = [
                        ppool.tile([P, 16, N], f8, tag=f"pl{q}", name=f"pl{q}")
                        for q in range(3)
                    ]
                    for q in range(3):
                        nc.gpsimd.dma_start(
                            out=planes[q][:],
                            in_=nat3[q].rearrange("(kc p) j -> p kc j", p=P),
                        )
                    def gemm(qi, rhs_res, out_res):
                        """out = plane_qi^T @ rhs (DoubleRow fp8 chain).

                        out_res: SBUF [P, 16, N] f8 (normalized, x SCALE) or
                        None (final: evict bf16 to h2t with 1/(16*SCALE)).
                        """
                        plane = planes[qi]
                        for ms in range(16):
                            ps = [
                                gpsum.tile([P, 512], f32, tag=f"ps{ic}", name=f"ps{ic}")
                                for ic in range(4)
                            ]
                            for kp in range(8):
                                for ic in range(4):
                                    nc.tensor.matmul(
                                        ps[ic][:],
                                        lhsT=plane[
                                            :,
                                            2 * kp : 2 * kp + 2,
                                            ms * P : (ms + 1) * P,
                                        ],
                                        rhs=rhs_res[
                                            :,
                                            2 * kp : 2 * kp + 2,
                                            ic * 512 : (ic + 1) * 512,
                                        ],
                                        start=(kp == 0),
                                        stop=(kp == 7),
                                        perf_mode=DR,
                                    )
                            if out_res is not None:
                                dc = (ms * P) // 512
                                v = ms % 4
                                degp = npool.tile([P, 4], f32, tag="degp")
                                # zero diag in place + row-sum of masked tile
                                nc.vector.scalar_tensor_tensor(
                                    out=ps[dc][:],
                                    in0=ps[dc][:],
                                    scalar=1.0,
                                    in1=masks[:, v],
                                    op0=MUL,
                                    op1=MUL,
                                    accum_out=degp[:, dc : dc + 1],
                                )
                                for ic in range(4):
                                    if ic != dc:
                                        nc.vector.tensor_reduce(
                                            degp[:, ic : ic + 1],
                                            ps[ic][:],
                                            AX,
                                            ADD,
                                        )
                                degs = npool.tile([P, 1], f32, tag="degs")
                                nc.vector.tensor_reduce(degs[:], degp[:], AX, ADD)
                                nc.vector.tensor_scalar_mul(
                                    degs[:], degs[:], 1.0 / SCALE
                                )
                                dinv = npool.tile([P, 1], f32, tag="dinv")
                                nc.vector.reciprocal(dinv[:], degs[:])
                                for ic in range(4):
                                    nc.scalar.activation(
                                        out_res[
                                            :, ms, ic * 512 : (ic + 1) * 512
                                        ],
                                        ps[ic][:],
                                        COPY,
                                        scale=dinv[:],
                                    )
                            else:
                                stf = gpool.tile([P, N], bf16, tag="stf")
                                for ic in range(4):
                                    nc.scalar.activation(
                                        stf[:, ic * 512 : (ic + 1) * 512],
                                        ps[ic][:],
                                        COPY,
                                        scale=1.0 / (16.0 * SCALE),
                                    )
                                nc.sync.dma_start(
                                    out=h2t[ms * P : (ms + 1) * P, :],
                                    in_=stf[:],
                                )

                    # GEMM1: Ht = b^T a^T ; normalize -> Hnt
                    gemm(0, mv[0], mv[1])
                    # GEMM2: H't = g1^T Hnt ; normalize -> H'nt (reuse mv0)
                    mv0b = bigpool.tile([P, 16, N], f8, tag="mv0")
                    gemm(1, mv[1], mv0b)
                    # GEMM3: H''t = g2^T H'nt -> h2t bf16
                    gemm(2, mv0b, None)

            # ===== Phase 5: local symmetrize y = H'' + H''^T =====
            with tc.tile_pool(name="sym", bufs=4) as ypool:
                for ms in range(16):
                    srow = ypool.tile([P, N], bf16, tag="srow")
                    nc.gpsimd.dma_start(
                        out=srow[:], in_=h2t[ms * P : (ms + 1) * P, :]
                    )
                    tcol = ypool.tile([P, N], bf16, tag="tcol")
                    teng = nc.sync if ms % 2 == 0 else nc.scalar
                    teng.dma_start_transpose(
                        out=tcol[:], in_=h2t[:, ms * P : (ms + 1) * P]
                    )
                    yt = ypool.tile([P, N], bf16, tag="yt")
                    nc.vector.tensor_add(yt[:], srow[:], tcol[:])
                    nc.gpsimd.dma_start(
                        out=ysym[ms * P : (ms + 1) * P, :], in_=yt[:]
                    )

                # ===== Phase 6: sum over channels, keep own row band =====
                nc.gpsimd.collective_compute(
                    "ReduceScatter",
                    ADD,
                    replica_groups=[list(range(NCORES))],
                    ins=[ysym.opt()],
                    outs=[rs.opt()],
                )

                for hh in range(2):
                    tb = ypool.tile([P, N], bf16, tag="tb")
                    nc.sync.dma_start(
                        out=tb[:], in_=rs[hh * P : (hh + 1) * P, :]
                    )
                    tf = ypool.tile([P, N], f32, tag="tf")
                    nc.vector.tensor_copy(out=tf[:], in_=tb[:])
                    nc.scalar.dma_start(
                        out=out_ext[hh * P : (hh + 1) * P, :], in_=tf[:]
                    )

    nc.compile()
    return nc


def _get_program():
    global _PROGRAM
    if _PROGRAM is None:
        _PROGRAM = _build_program()
    return _PROGRAM


def _make_w4d(sws) -> np.ndarray:
    """DoubleRow block-diagonal mix weights [128, 2, 128].

    w4d[x'*8+e, ko, q*32 + ko2*16 + x] = sws[q][e] iff x==x' and ko==ko2.
    Output partition m = q*32 + ko*16 + x computes mix q for row 16*(B+ko)+x
    of the block pair starting at block B.
    """
    w4d = np.zeros((P, 2, P), np.float32)
    for q, sw in enumerate(sws):
        for ko in range(2):
            for x in range(16):
                w4d[x * 8 : (x + 1) * 8, ko, q * 32 + ko * 16 + x] = sw
    return w4d


def _prep_inputs(A, w1_0, w2_0, w_1, w_2):
    import ml_dtypes

    f8 = ml_dtypes.float8_e4m3
    swa = _softmax_rows(np.asarray(w1_0)).astype(np.float32)
    swb = _softmax_rows(np.asarray(w2_0)).astype(np.float32)
    sg1 = _softmax_rows(np.asarray(w_1)).astype(np.float32)
    sg2 = _softmax_rows(np.asarray(w_2)).astype(np.float32)

    a8 = np.asarray(A, dtype=np.float32)[0].astype(f8)  # [k, j, e]
    # At3[b, (k16 e), j] = A[16b+k16, j, e]
    at3 = np.ascontiguousarray(a8.transpose(0, 2, 1)).reshape(P, P, N)
    in_maps = []
    for c in range(NCORES):
        w4d = _make_w4d([swa[c], swb[c], sg1[c], sg2[c]]).astype(f8)
        in_maps.append({"At3": at3, "w4d": w4d})
    return in_maps


def kernel(A, w1_0, w2_0, w_1, w_2):
    from concourse.bass_utils import run_bass_kernel_spmd

    in_maps = _prep_inputs(A, w1_0, w2_0, w_1, w_2)
    nc = _get_program()
    res = run_bass_kernel_spmd(nc, in_maps, list(range(NCORES)))
    return np.concatenate(
        [np.asarray(res.results[c]["out"], dtype=np.float32) for c in range(NCORES)],
        axis=0,
    )


# revision 23
# speedup vs baseline: 1.1044x; 1.1044x over previous
"""GTN (graph transformer network) meta-path kernel for TRN2, 8 NeuronCores.

Math (reference):
    Ap = A transposed to [E, N, N]
    a  = sum_e softmax(w1_0)[c,e] * Ap[e]      (per channel c)
    b  = sum_e softmax(w2_0)[c,e] * Ap[e]
    H  = a @ b
    twice:  H = normalize(H) @ gtconv(Ap, w)   (normalize = zero diag, col-scale)
    out = symmetrized mean over channels.

Sharding: channel-parallel — core c computes channel c end to end, then the
symmetrized per-channel result is summed with one bf16 ReduceScatter; core c
returns output rows [256c:256c+256] and the host concatenates the bands.

On-device formulation works with TRANSPOSED intermediates Ht = H^T so that
 - every GEMM's moving operand is the previous GEMM's output as-is,
 - normalization becomes row sums (free-dim reduce) + per-partition scale.

All GEMMs run in fp8(e4m3) DoubleRow perf mode (256-deep contraction per
pass, ~1.7x bf16 PE throughput). The normalized operand is scaled by
SCALE=1024 when quantized to fp8 (entries land ~0.5); normalization is
scale-invariant so the factor self-cancels at the next normalize, and the
final GEMM's output stage divides it back out along with the channel mean.

The edge-type mix contracts e on SBUF partitions: the host feeds A once in
fp8 layout At3[b, (k16 e), j] and a block-diagonal DoubleRow weight computes
all four mixes for a PAIR of 16-row blocks per pass. PSUM tiles are copied
full-width to bf16 staging; the a-plane rows go to a bf16 DRAM plane (read
back with DMA-transpose to build GEMM1's moving operand a^T) and the
b/g1/g2 rows go to fp8 DRAM planes via one casting SWDGE DMA per block
group. GEMM stationaries are SBUF-resident for the whole run.
"""

import numpy as np

N = 2048
E = 8
C = 8
P = 128
NCORES = 8
SCALE = 1024.0

_PROGRAM = None


def _softmax_rows(w: np.ndarray) -> np.ndarray:
    """w: [C, E, 1, 1] -> softmax over E, float64 precision, returns [C, E]."""
    x = w.reshape(C, E).astype(np.float64)
    x = x - x.max(axis=1, keepdims=True)
    ex = np.exp(x)
    return ex / ex.sum(axis=1, keepdims=True)


def _build_program():
    import concourse.bacc as bacc
    import concourse.mybir as mybir
    import concourse.tile as tile
    from concourse.masks import make_identity

    f32 = mybir.dt.float32
    bf16 = mybir.dt.bfloat16
    f8 = mybir.dt.float8e4
    AX = mybir.AxisListType.X
    MUL = mybir.AluOpType.mult
    ADD = mybir.AluOpType.add
    NE = mybir.AluOpType.not_equal
    COPY = mybir.ActivationFunctionType.Copy
    DR = mybir.MatmulPerfMode.DoubleRow

    nc = bacc.Bacc("TRN2")
    A3_ext = nc.dram_tensor("At3", [P, P, N], f8, kind="ExternalInput")
    w4_ext = nc.dram_tensor("w4d", [P, 2, P], f8, kind="ExternalInput")
    out_ext = nc.dram_tensor("out", [2 * P, N], f32, kind="ExternalOutput")

    with tile.TileContext(nc) as tc:
        with (
            tc.tile_pool(name="dram", bufs=1, space="DRAM") as dpool,
            tc.tile_pool(name="const", bufs=1) as cpool,
        ):
            aplane = dpool.tile([N, N], bf16)        # 'a' mix, natural [i, j]
            nat3 = dpool.tile([3, N, N], f8)         # b/g1/g2 mixes, natural
            ysym = dpool.tile([N, N], bf16)          # H'' + H''^T (per channel)
            rs = dpool.tile([2 * P, N], bf16)        # ReduceScatter shard

            # --- constants ---
            w4_sb = cpool.tile([P, 2, P], f8)
            nc.sync.dma_start(out=w4_sb[:], in_=w4_ext[:])
            identb = cpool.tile([P, P], bf16)
            make_identity(nc, identb[:])
            # diag masks: masks[:, v, y] = 0 where y == p + v*128 else 1
            masks = cpool.tile([P, 4, 512], f32)
            nc.gpsimd.memset(masks[:], 1.0)
            for v in range(4):
                nc.gpsimd.affine_select(
                    out=masks[:, v],
                    in_=masks[:, v],
                    compare_op=NE,
                    fill=0.0,
                    base=v * P,
                    pattern=[[-1, 512]],
                    channel_multiplier=1,
                )

            with tc.tile_pool(name="big", bufs=1) as bigpool:
                mv = [
                    bigpool.tile([P, 16, N], f8, tag="mv0", name="mva"),
                    bigpool.tile([P, 16, N], f8, tag="mv1", name="mvb"),
                ]

                # ======= Phase 1: all four mixes, DoubleRow block pairs ====
                with (
                    tc.tile_pool(name="mix", bufs=3) as mpool,
                    tc.tile_pool(name="mixst", bufs=3) as spool,
                    tc.tile_pool(name="mvt", bufs=2) as tpool,
                    tc.tile_pool(name="mixps", bufs=1, space="PSUM") as mpsum,
                ):
                    for ld4 in range(32):
                        a3t = mpool.tile([P, 4, N], f8, tag="a3t")
                        nc.sync.dma_start(
                            out=a3t[:],
                            in_=A3_ext[4 * ld4 : 4 * ld4 + 4].rearrange(
                                "b p j -> p b j"
                            ),
                        )
                        st8 = spool.tile([P, 2, N], f8, tag="st8")
                        sta = spool.tile([32, 2, N], bf16, tag="sta")
                        for h in range(2):
                            for jc in range(4):
                                pm = mpsum.tile(
                                    [P, 512], f32, tag=f"pm{h}{jc}", name="pm"
                                )
                                nc.tensor.matmul(
                                    pm[:],
                                    lhsT=w4_sb[:],
                                    rhs=a3t[
                                        :,
                                        2 * h : 2 * h + 2,
                                        jc * 512 : (jc + 1) * 512,
                                    ],
                                    start=True,
                                    stop=True,
                                    perf_mode=DR,
                                )
                                nc.vector.tensor_copy(
                                    out=st8[:, h, jc * 512 : (jc + 1) * 512],
                                    in_=pm[:],
                                )
                                nc.scalar.copy(
                                    sta[:, h, jc * 512 : (jc + 1) * 512],
                                    pm[0:32, :],
                                )
                        # a rows -> bf16 plane (1 DMA); SBUF AP stays
                        # partition-outermost, DRAM side reordered to match
                        nc.scalar.dma_start(
                            out=aplane[64 * ld4 : 64 * ld4 + 64, :].rearrange(
                                "(h r) j -> r h j", h=2
                            ),
                            in_=sta[:],
                        )
                        # b/g1/g2 rows -> fp8 planes (3 DMAs)
                        for q in range(3):
                            eng = (nc.sync, nc.gpsimd, nc.scalar)[q]
                            eng.dma_start(
                                out=nat3[
                                    q, 64 * ld4 : 64 * ld4 + 64, :
                                ].rearrange("(h r) j -> r h j", h=2),
                                in_=st8[32 * (q + 1) : 32 * (q + 2)],
                            )
                    # mv0 = a^T: DMA-transpose columns, cast bf16 -> fp8
                    for kc in range(16):
                        mt = tpool.tile([P, N], bf16, tag="mt")
                        teng = nc.sync if kc % 2 == 0 else nc.scalar
                        teng.dma_start_transpose(
                            out=mt[:], in_=aplane[:, kc * P : (kc + 1) * P]
                        )
                        if kc % 2 == 0:
                            nc.vector.tensor_copy(out=mv[0][:, kc, :], in_=mt[:])
                        else:
                            nc.scalar.copy(mv[0][:, kc, :], mt[:])
                # =========== Phases 2-4: three chained GEMMs ===========
                with (
                    tc.tile_pool(name="plns", bufs=1) as ppool,
                    tc.tile_pool(name="gw", bufs=2) as gpool,
                    tc.tile_pool(name="nrm", bufs=4) as npool,
                ):
                    # resident stationary plane loads (b blocks GEMM1;
                    # g1/g2 overlap GEMM1/2)
                    planes = [
                        ppool.tile([P, 16, N], f8, tag=f"pl{q}", name=f"pl{q}")
                        for q in range(3)
                    ]
                    for q in range(3):
                        nc.gpsimd.dma_start(
                            out=planes[q][:],
                            in_=nat3[q].rearrange("(kc p) j -> p kc j", p=P),
                        )
                    def mm_block(plane, rhs_res, ps, ms):
                        for kp in range(8):
                            for ic in range(4):
                                nc.tensor.matmul(
                                    ps[ic][:],
                                    lhsT=plane[
                                        :,
                                        2 * kp : 2 * kp + 2,
                                        ms * P : (ms + 1) * P,
                                    ],
                                    rhs=rhs_res[
                                        :,
                                        2 * kp : 2 * kp + 2,
                                        ic * 512 : (ic + 1) * 512,
                                    ],
                                    start=(kp == 0),
                                    stop=(kp == 7),
                                    perf_mode=DR,
                                )

                    def gemm(qi, rhs_res, out_res):
                        """out = norm(plane_qi^T @ rhs) * SCALE, fp8."""
                        plane = planes[qi]
                        with tc.tile_pool(
                            name=f"gps{qi}", bufs=2, space="PSUM"
                        ) as gpsum:
                            for ms in range(16):
                                ps = [
                                    gpsum.tile(
                                        [P, 512], f32,
                                        tag=f"ps{ic}", name=f"ps{ic}",
                                    )
                                    for ic in range(4)
                                ]
                                mm_block(plane, rhs_res, ps, ms)
                                dc = (ms * P) // 512
                                v = ms % 4
                                degp = npool.tile([P, 4], f32, tag="degp")
                                # zero diag in place + row-sum of masked tile
                                nc.vector.scalar_tensor_tensor(
                                    out=ps[dc][:],
                                    in0=ps[dc][:],
                                    scalar=1.0,
                                    in1=masks[:, v],
                                    op0=MUL,
                                    op1=MUL,
                                    accum_out=degp[:, dc : dc + 1],
                                )
                                for ic in range(4):
                                    if ic != dc:
                                        nc.vector.tensor_reduce(
                                            degp[:, ic : ic + 1],
                                            ps[ic][:],
                                            AX,
                                            ADD,
                                        )
                                degs = npool.tile([P, 1], f32, tag="degs")
                                nc.vector.tensor_reduce(
                                    degs[:], degp[:], AX, ADD
                                )
                                nc.vector.tensor_scalar_mul(
                                    degs[:], degs[:], 1.0 / SCALE
                                )
                                dinv = npool.tile([P, 1], f32, tag="dinv")
                                nc.vector.reciprocal(dinv[:], degs[:])
                                for ic in range(4):
                                    nc.scalar.activation(
                                        out_res[
                                            :, ms, ic * 512 : (ic + 1) * 512
                                        ],
                                        ps[ic][:],
                                        COPY,
                                        scale=dinv[:],
                                    )

                    # GEMM1: Ht = b^T a^T ; normalize -> Hnt
                    gemm(0, mv[0], mv[1])
                    # GEMM2: H't = g1^T Hnt ; normalize -> H'nt (reuse mv0)
                    mv0b = bigpool.tile([P, 16, N], f8, tag="mv0")
                    gemm(1, mv[1], mv0b)

                    # GEMM3: H''t = g2^T H'nt, fused with the symmetrize:
                    # each output band is also PE-transposed block-by-block
                    # and accumulated into SBUF-resident y = H'' + H''^T
                    # (reusing the dead b/g1 plane slots). Region (band c,
                    # col-window m) gets its transposed term at iteration m
                    # and its direct term at iteration c; whichever comes
                    # first is a copy, the other an add.
                    yslots = [
                        ppool.tile([P, 8, N], bf16, tag="pl0", name="yA"),
                        ppool.tile([P, 8, N], bf16, tag="pl1", name="yB"),
                    ]

                    with (
                        tc.tile_pool(name="gps2", bufs=1, space="PSUM") as g3ps,
                        tc.tile_pool(name="tps", bufs=1, space="PSUM") as tpsp,
                    ):
                        for ms in range(16):
                            msl = slice(ms * P, (ms + 1) * P)
                            ps = [
                                g3ps.tile(
                                    [P, 512], f32, tag=f"ps{ic}", name=f"ps{ic}"
                                )
                                for ic in range(4)
                            ]
                            mm_block(planes[2], mv0b, ps, ms)
                            stf = gpool.tile([P, N], bf16, tag="stf")
                            for ic in range(4):
                                eng = nc.scalar if ic % 2 == 0 else nc.vector
                                if ic % 2 == 0:
                                    eng.activation(
                                        stf[:, ic * 512 : (ic + 1) * 512],
                                        ps[ic][:],
                                        COPY,
                                        scale=1.0 / (16.0 * SCALE),
                                    )
                                else:
                                    eng.tensor_scalar_mul(
                                        stf[:, ic * 512 : (ic + 1) * 512],
                                        ps[ic][:],
                                        1.0 / (16.0 * SCALE),
                                    )
                            # direct term: row band ms (copy right of the
                            # diagonal block incl., add left of it)
                            if ms > 0:
                                nc.vector.tensor_add(
                                    yslots[ms // 8][:, ms % 8, 0 : ms * P],
                                    yslots[ms // 8][:, ms % 8, 0 : ms * P],
                                    stf[:, 0 : ms * P],
                                )
                            nc.vector.tensor_copy(
                                out=yslots[ms // 8][:, ms % 8, ms * P : N],
                                in_=stf[:, ms * P : N],
                            )
                            # transposed terms: bands c = 0..15, col window ms
                            for ic in range(4):
                                tps = tpsp.tile(
                                    [P, 512], bf16, tag=f"tp{ic}", name=f"tps{ic}"
                                )
                                for g in range(4):
                                    c = 4 * ic + g
                                    nc.tensor.transpose(
                                        tps[:, g * P : (g + 1) * P],
                                        stf[:, c * P : (c + 1) * P],
                                        identb[:],
                                    )
                                tv = tps[:].rearrange("p (g k) -> p g k", g=4)
                                cs = [4 * ic + g for g in range(4)]
                                nadd = sum(1 for c in cs if c <= ms)
                                sl = yslots[ic // 2]
                                cb = (4 * ic) % 8
                                if nadd:
                                    nc.vector.tensor_add(
                                        sl[:, cb : cb + nadd, msl],
                                        sl[:, cb : cb + nadd, msl],
                                        tv[:, 0:nadd],
                                    )
                                if nadd < 4:
                                    nc.vector.tensor_copy(
                                        out=sl[:, cb + nadd : cb + 4, msl],
                                        in_=tv[:, nadd:4],
                                    )

                    # evict y to DRAM for the collective
                    nc.sync.dma_start(
                        out=ysym[0:1024, :].rearrange("(u p) j -> p u j", p=P),
                        in_=yslots[0][:],
                    )
                    nc.scalar.dma_start(
                        out=ysym[1024:2048, :].rearrange(
                            "(u p) j -> p u j", p=P
                        ),
                        in_=yslots[1][:],
                    )

            # ===== Phase 6: sum over channels, keep own row band =====
            with tc.tile_pool(name="sym", bufs=2) as ypool:
                nc.gpsimd.collective_compute(
                    "ReduceScatter",
                    ADD,
                    replica_groups=[list(range(NCORES))],
                    ins=[ysym.opt()],
                    outs=[rs.opt()],
                )

                for hh in range(2):
                    tb = ypool.tile([P, N], bf16, tag="tb")
                    nc.sync.dma_start(
                        out=tb[:], in_=rs[hh * P : (hh + 1) * P, :]
                    )
                    tf = ypool.tile([P, N], f32, tag="tf")
                    nc.vector.tensor_copy(out=tf[:], in_=tb[:])
                    nc.scalar.dma_start(
                        out=out_ext[hh * P : (hh + 1) * P, :], in_=tf[:]
                    )

    nc.compile()
    return nc


def _get_program():
    global _PROGRAM
    if _PROGRAM is None:
        _PROGRAM = _build_program()
    return _PROGRAM


def _make_w4d(sws) -> np.ndarray:
    """DoubleRow block-diagonal mix weights [128, 2, 128].

    w4d[x'*8+e, ko, q*32 + ko2*16 + x] = sws[q][e] iff x==x' and ko==ko2.
    Output partition m = q*32 + ko*16 + x computes mix q for row 16*(B+ko)+x
    of the block pair starting at block B.
    """
    w4d = np.zeros((P, 2, P), np.float32)
    for q, sw in enumerate(sws):
        for ko in range(2):
            for x in range(16):
                w4d[x * 8 : (x + 1) * 8, ko, q * 32 + ko * 16 + x] = sw
    return w4d


def _prep_inputs(A, w1_0, w2_0, w_1, w_2):
    import ml_dtypes

    f8 = ml_dtypes.float8_e4m3
    swa = _softmax_rows(np.asarray(w1_0)).astype(np.float32)
    swb = _softmax_rows(np.asarray(w2_0)).astype(np.float32)
    sg1 = _softmax_rows(np.asarray(w_1)).astype(np.float32)
    sg2 = _softmax_rows(np.asarray(w_2)).astype(np.float32)

    a8 = np.asarray(A, dtype=np.float32)[0].astype(f8)  # [k, j, e]
    # At3[b, (k16 e), j] = A[16b+k16, j, e]
    at3 = np.ascontiguousarray(a8.transpose(0, 2, 1)).reshape(P, P, N)
    in_maps = []
    for c in range(NCORES):
        w4d = _make_w4d([swa[c], swb[c], sg1[c], sg2[c]]).astype(f8)
        in_maps.append({"At3": at3, "w4d": w4d})
    return in_maps


def kernel(A, w1_0, w2_0, w_1, w_2):
    from concourse.bass_utils import run_bass_kernel_spmd

    in_maps = _prep_inputs(A, w1_0, w2_0, w_1, w_2)
    nc = _get_program()
    res = run_bass_kernel_spmd(nc, in_maps, list(range(NCORES)))
    return np.concatenate(
        [np.asarray(res.results[c]["out"], dtype=np.float32) for c in range(NCORES)],
        axis=0,
    )


# revision 28
# speedup vs baseline: 1.1300x; 1.0232x over previous
"""GTN (graph transformer network) meta-path kernel for TRN2, 8 NeuronCores.

Math (reference):
    Ap = A transposed to [E, N, N]
    a  = sum_e softmax(w1_0)[c,e] * Ap[e]      (per channel c)
    b  = sum_e softmax(w2_0)[c,e] * Ap[e]
    H  = a @ b
    twice:  H = normalize(H) @ gtconv(Ap, w)   (normalize = zero diag, col-scale)
    out = symmetrized mean over channels.

Sharding: channel-parallel — core c computes channel c end to end, then the
symmetrized per-channel result is summed with one bf16 ReduceScatter; core c
returns output rows [256c:256c+256] and the host concatenates the bands.

On-device formulation works with TRANSPOSED intermediates Ht = H^T so that
 - every GEMM's moving operand is the previous GEMM's output as-is,
 - normalization becomes row sums (free-dim reduce) + per-partition scale.

All GEMMs run in fp8(e4m3) DoubleRow perf mode (256-deep contraction per
pass, ~1.7x bf16 PE throughput). The normalized operand is scaled by
SCALE=1024 when quantized to fp8 (entries land ~0.5); normalization is
scale-invariant so the factor self-cancels at the next normalize, and the
final GEMM's output stage divides it back out along with the channel mean.

The edge-type mix contracts e on SBUF partitions: the host feeds A once in
fp8 layout At3[b, (k16 e), j] and a block-diagonal DoubleRow weight computes
all four mixes for a PAIR of 16-row blocks per pass. PSUM tiles are copied
full-width to bf16 staging; the a-plane rows go to a bf16 DRAM plane (read
back with DMA-transpose to build GEMM1's moving operand a^T) and the
b/g1/g2 rows go to fp8 DRAM planes via one casting SWDGE DMA per block
group. GEMM stationaries are SBUF-resident for the whole run.
"""

import numpy as np

N = 2048
E = 8
C = 8
P = 128
NCORES = 8
SCALE = 1024.0

_PROGRAM = None


def _softmax_rows(w: np.ndarray) -> np.ndarray:
    """w: [C, E, 1, 1] -> softmax over E, float64 precision, returns [C, E]."""
    x = w.reshape(C, E).astype(np.float64)
    x = x - x.max(axis=1, keepdims=True)
    ex = np.exp(x)
    return ex / ex.sum(axis=1, keepdims=True)


def _build_program():
    import concourse.bacc as bacc
    import concourse.mybir as mybir
    import concourse.tile as tile
    from concourse.masks import make_identity

    f32 = mybir.dt.float32
    bf16 = mybir.dt.bfloat16
    f8 = mybir.dt.float8e4
    AX = mybir.AxisListType.X
    MUL = mybir.AluOpType.mult
    ADD = mybir.AluOpType.add
    NE = mybir.AluOpType.not_equal
    COPY = mybir.ActivationFunctionType.Copy
    DR = mybir.MatmulPerfMode.DoubleRow

    nc = bacc.Bacc("TRN2")
    A3_ext = nc.dram_tensor("At3", [P, P, N], f8, kind="ExternalInput")
    w4_ext = nc.dram_tensor("w4d", [P, 2, P], f8, kind="ExternalInput")
    out_ext = nc.dram_tensor("out", [2 * P, N], f32, kind="ExternalOutput")

    with tile.TileContext(nc) as tc:
        with (
            tc.tile_pool(name="dram", bufs=1, space="DRAM") as dpool,
            tc.tile_pool(name="const", bufs=1) as cpool,
        ):
            aplane = dpool.tile([N, N], bf16)        # 'a' mix, natural [i, j]
            ysym = dpool.tile([N, N], bf16)          # H'' + H''^T (per channel)
            rs = dpool.tile([2 * P, N], bf16)        # ReduceScatter shard

            # --- constants ---
            w4_sb = cpool.tile([P, 2, P], f8)
            nc.sync.dma_start(out=w4_sb[:], in_=w4_ext[:])
            identb = cpool.tile([P, P], bf16)
            make_identity(nc, identb[:])
            # diag masks: masks[:, v, y] = 0 where y == p + v*128 else 1
            masks = cpool.tile([P, 4, 512], bf16)
            nc.gpsimd.memset(masks[:], 1.0)
            for v in range(4):
                nc.gpsimd.affine_select(
                    out=masks[:, v],
                    in_=masks[:, v],
                    compare_op=NE,
                    fill=0.0,
                    base=v * P,
                    pattern=[[-1, 512]],
                    channel_multiplier=1,
                )

            with tc.tile_pool(name="big", bufs=1) as bigpool:
                mv = [
                    bigpool.tile([P, 16, N], f8, tag="mv0", name="mva"),
                    bigpool.tile([P, 16, N], f8, tag="mv1", name="mvb"),
                ]
                # GEMM stationaries, SBUF-resident for the whole run; filled
                # straight from the phase-1 staging tiles with partition-
                # shuffling SBUF->SBUF DMAs (no DRAM round-trip).
                planes = [
                    bigpool.tile([P, 16, N], f8, tag=f"pl{q}", name=f"pl{q}")
                    for q in range(3)
                ]

                # ======= Phase 1: all four mixes, DoubleRow block pairs ====
                with (
                    tc.tile_pool(name="mix", bufs=2) as mpool,
                    tc.tile_pool(name="mixst", bufs=2) as spool,
                    tc.tile_pool(name="mixps", bufs=1, space="PSUM") as mpsum,
                ):
                    for ld4 in range(32):
                        a3t = mpool.tile([P, 4, N], f8, tag="a3t")
                        nc.sync.dma_start(
                            out=a3t[:],
                            in_=A3_ext[4 * ld4 : 4 * ld4 + 4].rearrange(
                                "b p j -> p b j"
                            ),
                        )
                        st8 = spool.tile([P, 2, N], f8, tag="st8")
                        sta = spool.tile([32, 2, N], bf16, tag="sta")
                        for h in range(2):
                            for jc in range(4):
                                pm = mpsum.tile(
                                    [P, 512], f32, tag=f"pm{h}{jc}", name="pm"
                                )
                                nc.tensor.matmul(
                                    pm[:],
                                    lhsT=w4_sb[:],
                                    rhs=a3t[
                                        :,
                                        2 * h : 2 * h + 2,
                                        jc * 512 : (jc + 1) * 512,
                                    ],
                                    start=True,
                                    stop=True,
                                    perf_mode=DR,
                                )
                                nc.vector.tensor_copy(
                                    out=st8[:, h, jc * 512 : (jc + 1) * 512],
                                    in_=pm[:],
                                )
                                nc.scalar.copy(
                                    sta[:, h, jc * 512 : (jc + 1) * 512],
                                    pm[0:32, :],
                                )
                        # a rows -> bf16 plane (1 DMA); SBUF AP stays
                        # partition-outermost, DRAM side reordered to match
                        nc.scalar.dma_start(
                            out=aplane[64 * ld4 : 64 * ld4 + 64, :].rearrange(
                                "(h r) j -> r h j", h=2
                            ),
                            in_=sta[:],
                        )
                        # b/g1/g2 rows into the resident planes: partition
                        # shuffle (32q+32 -> 64*(ld4%2)+32h), chunk ld4//2
                        for h in range(2):
                            po = 64 * (ld4 % 2) + 32 * h
                            for q in range(3):
                                eng = (nc.sync, nc.gpsimd, nc.sync)[q]
                                eng.dma_start(
                                    out=planes[q][
                                        po : po + 32, ld4 // 2, :
                                    ],
                                    in_=st8[32 * (q + 1) : 32 * (q + 2), h, :],
                                )
                # mv0 = a^T: DMA-transpose columns, cast bf16 -> fp8
                with tc.tile_pool(name="mvt", bufs=2) as tpool:
                    for kc in range(16):
                        mt = tpool.tile([P, N], bf16, tag="mt")
                        teng = nc.sync if kc % 2 == 0 else nc.scalar
                        teng.dma_start_transpose(
                            out=mt[:], in_=aplane[:, kc * P : (kc + 1) * P]
                        )
                        if kc % 2 == 0:
                            nc.vector.tensor_copy(out=mv[0][:, kc, :], in_=mt[:])
                        else:
                            nc.scalar.copy(mv[0][:, kc, :], mt[:])
                # =========== Phases 2-4: three chained GEMMs ===========
                with (
                    tc.tile_pool(name="gw", bufs=2) as gpool,
                    tc.tile_pool(name="nrm", bufs=4) as npool,
                ):
                    def mm_block(plane, rhs_res, ps, ms):
                        for kp in range(8):
                            for ic in range(4):
                                nc.tensor.matmul(
                                    ps[ic][:],
                                    lhsT=plane[
                                        :,
                                        2 * kp : 2 * kp + 2,
                                        ms * P : (ms + 1) * P,
                                    ],
                                    rhs=rhs_res[
                                        :,
                                        2 * kp : 2 * kp + 2,
                                        ic * 512 : (ic + 1) * 512,
                                    ],
                                    start=(kp == 0),
                                    stop=(kp == 7),
                                    perf_mode=DR,
                                )

                    def gemm(qi, rhs_res, out_res):
                        """out = norm(plane_qi^T @ rhs) * SCALE, fp8."""
                        plane = planes[qi]
                        with tc.tile_pool(
                            name=f"gps{qi}", bufs=2, space="PSUM"
                        ) as gpsum:
                            for ms in range(16):
                                ps = [
                                    gpsum.tile(
                                        [P, 512], f32,
                                        tag=f"ps{ic}", name=f"ps{ic}",
                                    )
                                    for ic in range(4)
                                ]
                                mm_block(plane, rhs_res, ps, ms)
                                dc = (ms * P) // 512
                                v = ms % 4
                                degp = npool.tile([P, 4], f32, tag="degp")
                                # zero diag in place + row-sum of masked tile
                                nc.vector.scalar_tensor_tensor(
                                    out=ps[dc][:],
                                    in0=ps[dc][:],
                                    scalar=1.0,
                                    in1=masks[:, v],
                                    op0=MUL,
                                    op1=MUL,
                                    accum_out=degp[:, dc : dc + 1],
                                )
                                for ic in range(4):
                                    if ic != dc:
                                        nc.vector.tensor_reduce(
                                            degp[:, ic : ic + 1],
                                            ps[ic][:],
                                            AX,
                                            ADD,
                                        )
                                degs = npool.tile([P, 1], f32, tag="degs")
                                nc.vector.tensor_reduce(
                                    degs[:], degp[:], AX, ADD
                                )
                                nc.vector.tensor_scalar_mul(
                                    degs[:], degs[:], 1.0 / SCALE
                                )
                                dinv = npool.tile([P, 1], f32, tag="dinv")
                                nc.vector.reciprocal(dinv[:], degs[:])
                                for ic in range(4):
                                    nc.scalar.activation(
                                        out_res[
                                            :, ms, ic * 512 : (ic + 1) * 512
                                        ],
                                        ps[ic][:],
                                        COPY,
                                        scale=dinv[:],
                                    )

                    # GEMM1: Ht = b^T a^T ; normalize -> Hnt
                    gemm(0, mv[0], mv[1])
                    # GEMM2: H't = g1^T Hnt ; normalize -> H'nt (reuse mv0)
                    mv0b = bigpool.tile([P, 16, N], f8, tag="mv0")
                    gemm(1, mv[1], mv0b)

                    # GEMM3: H''t = g2^T H'nt, fused with the symmetrize:
                    # each output band is also PE-transposed block-by-block
                    # and accumulated into SBUF-resident y = H'' + H''^T
                    # (reusing the dead b/g1 plane slots). Region (band c,
                    # col-window m) gets its transposed term at iteration m
                    # and its direct term at iteration c; whichever comes
                    # first is a copy, the other an add.
                    yslots = [
                        bigpool.tile([P, 8, N], bf16, tag="pl0", name="yA"),
                        bigpool.tile([P, 8, N], bf16, tag="pl1", name="yB"),
                    ]

                    with (
                        tc.tile_pool(name="gps2", bufs=1, space="PSUM") as g3ps,
                        tc.tile_pool(name="tps", bufs=1, space="PSUM") as tpsp,
                    ):
                        for ms in range(16):
                            msl = slice(ms * P, (ms + 1) * P)
                            ps = [
                                g3ps.tile(
                                    [P, 512], f32, tag=f"ps{ic}", name=f"ps{ic}"
                                )
                                for ic in range(4)
                            ]
                            mm_block(planes[2], mv0b, ps, ms)
                            stf = gpool.tile([P, N], bf16, tag="stf")
                            for ic in range(4):
                                eng = nc.scalar if ic % 2 == 0 else nc.vector
                                if ic % 2 == 0:
                                    eng.activation(
                                        stf[:, ic * 512 : (ic + 1) * 512],
                                        ps[ic][:],
                                        COPY,
                                        scale=1.0 / (16.0 * SCALE),
                                    )
                                else:
                                    eng.tensor_scalar_mul(
                                        stf[:, ic * 512 : (ic + 1) * 512],
                                        ps[ic][:],
                                        1.0 / (16.0 * SCALE),
                                    )
                            # direct term: row band ms (copy right of the
                            # diagonal block incl., add left of it)
                            if ms > 0:
                                nc.vector.tensor_add(
                                    yslots[ms // 8][:, ms % 8, 0 : ms * P],
                                    yslots[ms // 8][:, ms % 8, 0 : ms * P],
                                    stf[:, 0 : ms * P],
                                )
                            nc.vector.tensor_copy(
                                out=yslots[ms // 8][:, ms % 8, ms * P : N],
                                in_=stf[:, ms * P : N],
                            )
                            # transposed terms: bands c = 0..15, col window ms
                            for ic in range(4):
                                tps = tpsp.tile(
                                    [P, 512], bf16, tag=f"tp{ic}", name=f"tps{ic}"
                                )
                                for g in range(4):
                                    c = 4 * ic + g
                                    nc.tensor.transpose(
                                        tps[:, g * P : (g + 1) * P],
                                        stf[:, c * P : (c + 1) * P],
                                        identb[:],
                                    )
                                tv = tps[:].rearrange("p (g k) -> p g k", g=4)
                                cs = [4 * ic + g for g in range(4)]
                                nadd = sum(1 for c in cs if c <= ms)
                                sl = yslots[ic // 2]
                                cb = (4 * ic) % 8
                                if nadd:
                                    nc.vector.tensor_add(
                                        sl[:, cb : cb + nadd, msl],
                                        sl[:, cb : cb + nadd, msl],
                                        tv[:, 0:nadd],
                                    )
                                if nadd < 4:
                                    nc.vector.tensor_copy(
                                        out=sl[:, cb + nadd : cb + 4, msl],
                                        in_=tv[:, nadd:4],
                                    )

                    # evict y to DRAM for the collective
                    nc.sync.dma_start(
                        out=ysym[0:1024, :].rearrange("(u p) j -> p u j", p=P),
                        in_=yslots[0][:],
                    )
                    nc.scalar.dma_start(
                        out=ysym[1024:2048, :].rearrange(
                            "(u p) j -> p u j", p=P
                        ),
                        in_=yslots[1][:],
                    )

            # ===== Phase 6: sum over channels, keep own row band =====
            with tc.tile_pool(name="sym", bufs=2) as ypool:
                nc.gpsimd.collective_compute(
                    "ReduceScatter",
                    ADD,
                    replica_groups=[list(range(NCORES))],
                    ins=[ysym.opt()],
                    outs=[rs.opt()],
                )

                for hh in range(2):
                    tb = ypool.tile([P, N], bf16, tag="tb")
                    nc.sync.dma_start(
                        out=tb[:], in_=rs[hh * P : (hh + 1) * P, :]
                    )
                    tf = ypool.tile([P, N], f32, tag="tf")
                    nc.vector.tensor_copy(out=tf[:], in_=tb[:])
                    nc.scalar.dma_start(
                        out=out_ext[hh * P : (hh + 1) * P, :], in_=tf[:]
                    )

    nc.compile()
    return nc


def _get_program():
    global _PROGRAM
    if _PROGRAM is None:
        _PROGRAM = _build_program()
    return _PROGRAM


def _make_w4d(sws) -> np.ndarray:
    """DoubleRow block-diagonal mix weights [128, 2, 128].

    w4d[x'*8+e, ko, q*32 + ko2*16 + x] = sws[q][e] iff x==x' and ko==ko2.
    Output partition m = q*32 + ko*16 + x computes mix q for row 16*(B+ko)+x
    of the block pair starting at block B.
    """
    w4d = np.zeros((P, 2, P), np.float32)
    for q, sw in enumerate(sws):
        for ko in range(2):
            for x in range(16):
                w4d[x * 8 : (x + 1) * 8, ko, q * 32 + ko * 16 + x] = sw
    return w4d


def _prep_inputs(A, w1_0, w2_0, w_1, w_2):
    import ml_dtypes

    f8 = ml_dtypes.float8_e4m3
    swa = _softmax_rows(np.asarray(w1_0)).astype(np.float32)
    swb = _softmax_rows(np.asarray(w2_0)).astype(np.float32)
    sg1 = _softmax_rows(np.asarray(w_1)).astype(np.float32)
    sg2 = _softmax_rows(np.asarray(w_2)).astype(np.float32)

    a8 = np.asarray(A, dtype=np.float32)[0].astype(f8)  # [k, j, e]
    # At3[b, (k16 e), j] = A[16b+k16, j, e]
    at3 = np.ascontiguousarray(a8.transpose(0, 2, 1)).reshape(P, P, N)
    in_maps = []
    for c in range(NCORES):
        w4d = _make_w4d([swa[c], swb[c], sg1[c], sg2[c]]).astype(f8)
        in_maps.append({"At3": at3, "w4d": w4d})
    return in_maps


def kernel(A, w1_0, w2_0, w_1, w_2):
    from concourse.bass_utils import run_bass_kernel_spmd

    in_maps = _prep_inputs(A, w1_0, w2_0, w_1, w_2)
    nc = _get_program()
    res = run_bass_kernel_spmd(nc, in_maps, list(range(NCORES)))
    return np.concatenate(
        [np.asarray(res.results[c]["out"], dtype=np.float32) for c in range(NCORES)],
        axis=0,
    )
